# revision 2
# baseline (speedup 1.0000x reference)
"""TRN2 Bass kernel for nn_MetaBaseline (DN4-style local-descriptor kNN).

Reference computation (per batch b):
  q = normalize(input1[b].reshape(75, 100, 640), axis=-1)      # query patches
  s = normalize(input2[b].reshape(2500, 640), axis=-1)         # support descs
  scores = q_patches @ s.T                                     # [7500, 2500]
  per way group g (columns [500g, 500g+500)): top-k (k=5) per row, mean,
  then sum over the 100 patches of each query -> out [75, 5].

Sharding: data-parallel over (b, query-quarter): 8 cores, each handles one
batch's quarter of queries (19 queries padded) with that batch's full
support replicated.

Per-core device program:
  - normalize support tiles (ACT square+accum, sqrt, DVE reciprocal, ACT scale)
  - PE-transpose q and s into [C-chunk, desc] layout (fp32 -> float32r SBUF)
  - scores: fp32r matmuls [128 patches x 500 descs] accumulated over 5
    C-chunks in PSUM (one bank per way group)
  - top-8 per patch/way via DVE max (reads PSUM), sum first k via strided
    tensor_reduce, scale by 1/(k*|q_patch|) (ACT, per-partition scalar)
  - per-query reduction over patches via a small fp32 indicator matmul
    accumulated across the 15 patch tiles in PSUM -> [19, 5]
"""
from contextlib import ExitStack

import numpy as np

import concourse.bass as bass  # noqa: F401  (engine namespaces live on the nc object)
import concourse.mybir as mybir
import concourse.tile as tile
from concourse import bacc
from concourse.bass_utils import run_bass_kernel_spmd
from concourse.masks import make_identity

# Problem geometry (hardcoded per contest rules)
B, Q, WAY, SHOT, H, W, C = 2, 75, 5, 5, 10, 10, 640
HW = H * W               # 100 patches per query / support image
NQ = 19                  # queries per core (4 cores x 19 = 76 >= 75)
MT = 15                  # patch M-tiles of 128 -> 1920 rows (1900 real)
PAD_P = MT * 128
NS = WAY * SHOT * HW     # 2500 support descriptors per batch
ST = 20                  # support tiles of 128 -> 2560 rows
PAD_S = ST * 128
KC = 5                   # C chunks of 128 (640 = 5*128)
P = 128
NW = SHOT * HW           # 500 support descriptors per way group
N_CORES = 8

_prog_cache: dict[int, object] = {}


def _build(k: int):
    """Build + compile the per-core SPMD program for neighbor_k == k."""
    assert 1 <= k <= 8, f"neighbor_k={k} not supported (need 1..8)"
    nc = bacc.Bacc("TRN2", target_bir_lowering=False, debug=False)
    f32 = mybir.dt.float32
    f32r = mybir.dt.float32r
    AF = mybir.ActivationFunctionType

    q_d = nc.dram_tensor("q", [PAD_P, C], f32, kind="ExternalInput").ap()
    s_d = nc.dram_tensor("s", [PAD_S, C], f32, kind="ExternalInput").ap()
    ind_d = nc.dram_tensor("ind", [P, MT * NQ], f32, kind="ExternalInput").ap()
    out_d = nc.dram_tensor("out", [NQ, WAY], f32, kind="ExternalOutput").ap()

    with tile.TileContext(nc) as tc:
        with ExitStack() as ctx:
            const = ctx.enter_context(tc.tile_pool(name="const", bufs=1))
            big = ctx.enter_context(tc.tile_pool(name="big", bufs=1))
            loads = ctx.enter_context(tc.tile_pool(name="loads", bufs=3))
            small = ctx.enter_context(tc.tile_pool(name="small", bufs=3))
            outp = ctx.enter_context(
                tc.tile_pool(name="outp", bufs=1, space="PSUM")
            )

            ident = const.tile([P, P], f32)
            make_identity(nc, ident)
            ind_sb = const.tile([P, MT * NQ], f32)
            nc.sync.dma_start(out=ind_sb, in_=ind_d)
            qinv = const.tile([P, MT], f32)

            s_T = [big.tile([P, PAD_S], f32r, tag=f"sT{c}", name=f"sT{c}") for c in range(KC)]
            q_T = [big.tile([P, PAD_P], f32r, tag=f"qT{c}", name=f"qT{c}") for c in range(KC)]
            out_ps = outp.tile([NQ, WAY], f32)

            # ---- phase A: support normalize + transpose ----
            with tc.tile_pool(name="tpsum", bufs=3, space="PSUM") as tpsum:
                for t in range(ST):
                    s_tile = loads.tile([P, C], f32, tag="s_tile")
                    nc.sync.dma_start(out=s_tile, in_=s_d[t * P:(t + 1) * P, :])
                    sq = loads.tile([P, C], f32, tag="sq")
                    ssum = small.tile([P, 1], f32, tag="ssum")
                    nc.scalar.activation(sq, s_tile, AF.Square, accum_out=ssum)
                    snrm = small.tile([P, 1], f32, tag="snrm")
                    nc.scalar.sqrt(snrm, ssum)
                    sinv = small.tile([P, 1], f32, tag="sinv")
                    nc.vector.reciprocal(sinv, snrm)
                    s_n = loads.tile([P, C], f32, tag="s_n")
                    nc.scalar.mul(s_n, s_tile, sinv)
                    for c in range(KC):
                        tps = tpsum.tile([P, P], f32, tag="tps")
                        nc.tensor.transpose(tps, s_n[:, c * P:(c + 1) * P], ident)
                        nc.scalar.copy(s_T[c][:, t * P:(t + 1) * P], tps)

                # ---- phase B: query norms + transpose ----
                for t in range(MT):
                    q_tile = loads.tile([P, C], f32, tag="q_tile")
                    nc.sync.dma_start(out=q_tile, in_=q_d[t * P:(t + 1) * P, :])
                    sq = loads.tile([P, C], f32, tag="sq")
                    qsum = small.tile([P, 1], f32, tag="ssum")
                    nc.scalar.activation(sq, q_tile, AF.Square, accum_out=qsum)
                    kn = small.tile([P, 1], f32, tag="snrm")
                    # sqrt(k^2 * sum(q^2)) = k * |q|
                    nc.scalar.activation(kn, qsum, AF.Sqrt, scale=float(k * k))
                    nc.vector.reciprocal(qinv[:, t:t + 1], kn)
                    for c in range(KC):
                        tps = tpsum.tile([P, P], f32, tag="tps")
                        nc.tensor.transpose(tps, q_tile[:, c * P:(c + 1) * P], ident)
                        nc.scalar.copy(q_T[c][:, t * P:(t + 1) * P], tps)

            # ---- phase C: scores + top-k + per-query reduction ----
            with tc.tile_pool(name="spsum", bufs=5, space="PSUM") as spsum:
                for m in range(MT):
                    pscs = [
                        spsum.tile([P, NW], f32, tag="psc", name=f"psc{m}_{_w}") for _w in range(WAY)
                    ]
                    for c in range(KC):
                        lhsT = q_T[c][:, m * P:(m + 1) * P]
                        for w in range(WAY):
                            nc.tensor.matmul(
                                pscs[w],
                                lhsT,
                                s_T[c][:, w * NW:(w + 1) * NW],
                                start=(c == 0),
                                stop=(c == KC - 1),
                            )
                    mx = small.tile([P, WAY * 8], f32, tag="mx")
                    for w in range(WAY):
                        nc.vector.max(mx[:, w * 8:(w + 1) * 8], pscs[w])
                    tsum = small.tile([P, WAY], f32, tag="tsum")
                    nc.vector.tensor_reduce(
                        tsum,
                        mx.rearrange("p (w j) -> p w j", w=WAY)[:, :, :k],
                        axis=mybir.AxisListType.X,
                        op=mybir.AluOpType.add,
                    )
                    scaled = small.tile([P, WAY], f32, tag="scaled")
                    nc.scalar.mul(scaled, tsum, qinv[:, m:m + 1])
                    nc.tensor.matmul(
                        out_ps,
                        ind_sb[:, m * NQ:(m + 1) * NQ],
                        scaled,
                        start=(m == 0),
                        stop=(m == MT - 1),
                    )
                out_sb = small.tile([NQ, WAY], f32, tag="out_sb")
                nc.scalar.copy(out_sb, out_ps)
                nc.sync.dma_start(out=out_d, in_=out_sb)

    nc.compile()
    return nc


def get_program(k: int):
    if k not in _prog_cache:
        _prog_cache[k] = _build(k)
    return _prog_cache[k]


def make_in_maps(input1: np.ndarray, input2: np.ndarray):
    """Shard full inputs into per-core input maps."""
    input1 = np.ascontiguousarray(np.asarray(input1), dtype=np.float32)
    input2 = np.ascontiguousarray(np.asarray(input2), dtype=np.float32)
    in_maps = []
    for core in range(N_CORES):
        b = core // 4
        qs = (core % 4) * NQ
        qe = min(Q, qs + NQ)
        nq = qe - qs
        qdat = input1[b].reshape(Q, HW, C)[qs:qe].reshape(-1, C)
        qfull = np.ones((PAD_P, C), np.float32)
        qfull[: nq * HW] = qdat
        sfull = np.ones((PAD_S, C), np.float32)
        sfull[:NS] = input2[b].reshape(NS, C)
        # indicator: patch row p of M-tile t belongs to query (t*128+p)//HW
        ind = np.zeros((P, MT * NQ), np.float32)
        g = np.arange(MT * P)
        j = g // HW
        valid = j < nq
        ind[g[valid] % P, (g[valid] // P) * NQ + j[valid]] = 1.0
        in_maps.append({"q": qfull, "s": sfull, "ind": ind})
    return in_maps


def gather_out(results) -> np.ndarray:
    out = np.zeros((B, Q, WAY), np.float32)
    for core in range(N_CORES):
        b = core // 4
        qs = (core % 4) * NQ
        n = min(Q, qs + NQ) - qs
        out[b, qs:qs + n] = results[core]["out"][:n]
    return out


def kernel(input1, input2, neighbor_k):
    k = int(np.asarray(neighbor_k))
    nc = get_program(k)
    in_maps = make_in_maps(input1, input2)
    res = run_bass_kernel_spmd(nc, in_maps, core_ids=list(range(N_CORES)))
    return gather_out(res.results)


# revision 12
# speedup vs baseline: 1.1333x; 1.1333x over previous
"""TRN2 Bass kernel for nn_MetaBaseline (DN4-style local-descriptor kNN).

Reference computation (per batch b):
  q = normalize(input1[b].reshape(75, 100, 640), axis=-1)      # query patches
  s = normalize(input2[b].reshape(2500, 640), axis=-1)         # support descs
  scores = q_patches @ s.T                                     # [7500, 2500]
  per way group g (columns [500g, 500g+500)): top-k per row, mean,
  then sum over the 100 patches of each query -> out [75, 5].

Sharding: data-parallel over (b, query-quarter): 8 cores, each handles one
batch's quarter of queries (19 queries padded) with that batch's full
support replicated.

Per-core device program. Engines execute in emission order, so emission is
software-pipelined. The score loop is WAY-OUTER: pass w only needs support
descriptor tiles 0..4w+3, so score matmuls start as soon as the first four
support tiles are normalized+transposed; the remaining support prep streams
in the background during passes 0-3, and query prep (norm chain, packed PE
transposes, float32r eviction) is folded into pass 0 one tile ahead.
Top-8 per (patch, way) via DVE max straight from the PSUM score bank
(bank freed immediately after); pass 4 finishes each patch tile with a
strided top-k tensor_reduce, ACT scale by 1/(k*|q_patch|), and a small
fp32 indicator matmul accumulating per-query sums in PSUM -> [19, 5].
"""
import os
from contextlib import ExitStack

import numpy as np

import concourse.bass as bass  # noqa: F401
import concourse.mybir as mybir
import concourse.tile as tile
from concourse import bacc
from concourse.bass_utils import run_bass_kernel_spmd

# Problem geometry (hardcoded per contest rules)
B, Q, WAY, SHOT, H, W, C = 2, 75, 5, 5, 10, 10, 640
HW = H * W               # 100 patches per query / support image
NQ = 19                  # queries per core (4 cores x 19 = 76 >= 75)
MT = 15                  # patch M-tiles of 128 -> 1920 rows (1900 real)
PAD_P = MT * 128
NS = WAY * SHOT * HW     # 2500 support descriptors per batch
ST = 20                  # support tiles of 128 -> 2560 rows
PAD_S = ST * 128
KC = 5                   # C chunks of 128 (640 = 5*128)
P = 128
NW = SHOT * HW           # 500 support descriptors per way group
N_CORES = 8
N_WARM = int(os.environ.get("N_WARM", "14"))

_prog_cache: dict[int, object] = {}


def _build(k: int):
    """Build + compile the per-core SPMD program for neighbor_k == k."""
    assert 1 <= k <= 8, f"neighbor_k={k} not supported (need 1..8)"
    nc = bacc.Bacc("TRN2", target_bir_lowering=False, debug=False)
    f32 = mybir.dt.float32
    f32r = mybir.dt.float32r
    AF = mybir.ActivationFunctionType

    q_d = nc.dram_tensor("q", [PAD_P, C], f32, kind="ExternalInput").ap()
    s_d = nc.dram_tensor("s", [PAD_S, C], f32, kind="ExternalInput").ap()
    ind_d = nc.dram_tensor("ind", [P, MT * NQ], f32, kind="ExternalInput").ap()
    ident_d = nc.dram_tensor("ident", [P, P], f32, kind="ExternalInput").ap()
    out_d = nc.dram_tensor("out", [NQ, WAY], f32, kind="ExternalOutput").ap()

    with tile.TileContext(nc) as tc:
        with ExitStack() as ctx:
            const = ctx.enter_context(tc.tile_pool(name="const", bufs=1))
            big = ctx.enter_context(tc.tile_pool(name="big", bufs=1))
            loads = ctx.enter_context(tc.tile_pool(name="loads", bufs=6))
            small = ctx.enter_context(tc.tile_pool(name="small", bufs=4))
            mxp = ctx.enter_context(tc.tile_pool(name="mxp", bufs=MT))
            outp = ctx.enter_context(
                tc.tile_pool(name="outp", bufs=1, space="PSUM")
            )
            tp4 = ctx.enter_context(
                tc.tile_pool(name="tp4", bufs=2, space="PSUM")
            )
            tp1 = ctx.enter_context(
                tc.tile_pool(name="tp1", bufs=2, space="PSUM")
            )
            spp = ctx.enter_context(
                tc.tile_pool(name="spp", bufs=3, space="PSUM")
            )

            ident = const.tile([P, P], f32)
            ind_sb = const.tile([P, MT * NQ], f32)
            qinv = const.tile([P, MT], f32)

            # chunk c of each transposed tensor has its own column band so a
            # packed 4-chunk PSUM bank evicts with one strided copy
            s_T = big.tile([P, KC * PAD_S], f32r, name="s_T")
            q_T = big.tile([P, KC * PAD_P], f32r, name="q_T")

            def sT(c):
                return s_T[:, c * PAD_S:(c + 1) * PAD_S]

            def qT(c):
                return q_T[:, c * PAD_P:(c + 1) * PAD_P]

            out_ps = outp.tile([NQ, WAY], f32)

            # ---- warmups: ACT tables + PE pipeline (no DMA deps) ----
            wtile = const.tile([P, P], f32, name="wtile")
            nc.gpsimd.memset(wtile, 1.0)
            wsum = small.tile([P, 1], f32, tag="ssum")
            wsq = small.tile([P, 1], f32, tag="snrm")
            nc.scalar.activation(
                wsq, wtile[:, 0:1], AF.Square, accum_out=wsum
            )
            nc.scalar.sqrt(wsq, wsum)
            wps = tp4.tile([P, 4 * P], f32, tag="tp4")
            for i in range(N_WARM):
                nc.tensor.transpose(
                    wps[:, (i % 4) * P:(i % 4 + 1) * P], wtile, wtile
                )

            nev = 0

            def evict(out_ap, src_ap):
                nonlocal nev
                if nev % 2 == 0:
                    nc.vector.tensor_copy(out_ap, src_ap)
                else:
                    nc.scalar.copy(out_ap, src_ap)
                nev += 1

            def transpose_evict(x, T_all, T_pad, t):
                """5 packed PE transposes of x into T_all's column bands."""
                psA = tp4.tile([P, 4 * P], f32, tag="tp4", name=f"psA_{t}")
                for c in range(4):
                    nc.tensor.transpose(
                        psA[:, c * P:(c + 1) * P],
                        x[:, c * P:(c + 1) * P], ident)
                psB = tp1.tile([P, P], f32, tag="tp1", name=f"psB_{t}")
                nc.tensor.transpose(psB, x[:, 4 * P:5 * P], ident)
                out_ap = T_all[:, :4 * T_pad].rearrange(
                    "p (c n) -> p c n", c=4
                )[:, :, t * P:(t + 1) * P]
                evict(out_ap, psA.rearrange("p (c n) -> p c n", c=4))
                evict(T_all[:, 4 * T_pad + t * P:4 * T_pad + (t + 1) * P], psB)

            xs_s = [None] * ST
            xs_q = [None] * MT

            def s_dma(t):
                x = loads.tile([P, C], f32, tag="x_tile", name=f"sx{t}")
                nc.sync.dma_start(out=x, in_=s_d[t * P:(t + 1) * P, :])
                xs_s[t] = x

            def q_dma(m):
                x = loads.tile([P, C], f32, tag="x_tile", name=f"qx{m}")
                nc.sync.dma_start(out=x, in_=q_d[m * P:(m + 1) * P, :])
                xs_q[m] = x

            def s_prep(t, scale_on_dve=False):
                x = xs_s[t]
                sq = loads.tile([P, C], f32, tag="sq", name=f"ssq{t}")
                ssum = small.tile([P, 1], f32, tag="ssum")
                nc.scalar.activation(sq, x, AF.Square, accum_out=ssum)
                snrm = small.tile([P, 1], f32, tag="snrm")
                nc.scalar.sqrt(snrm, ssum)
                sinv = small.tile([P, 1], f32, tag="sinv")
                nc.vector.reciprocal(sinv, snrm)
                s_n = loads.tile([P, C], f32, tag="s_n", name=f"sn{t}")
                if scale_on_dve:
                    nc.vector.tensor_scalar_mul(s_n, x, sinv)
                else:
                    nc.gpsimd.tensor_scalar_mul(s_n, x, sinv)
                transpose_evict(s_n, s_T, PAD_S, t)

            def q_prep(m):
                x = xs_q[m]
                sq = loads.tile([P, C], f32, tag="sq", name=f"qsq{m}")
                qsum = small.tile([P, 1], f32, tag="ssum")
                nc.scalar.activation(sq, x, AF.Square, accum_out=qsum)
                kn = small.tile([P, 1], f32, tag="snrm")
                # sqrt(k^2 * sum(q^2)) = k * |q|
                nc.scalar.activation(kn, qsum, AF.Sqrt, scale=float(k * k))
                nc.vector.reciprocal(qinv[:, m:m + 1], kn)
                transpose_evict(x, q_T, PAD_P, m)

            # ---- prologue: support tiles 0-3, queries 0-1 ----
            # DMA order: support first (its prep chain is the pace-setter),
            # then ident (first needed by real transposes), queries, ind.
            for t in range(4):
                s_dma(t)
            nc.sync.dma_start(out=ident, in_=ident_d)
            q_dma(0)
            q_dma(1)
            nc.sync.dma_start(out=ind_sb, in_=ind_d)
            next_s = [4]

            def s_dma_ahead(upto):
                while next_s[0] <= min(upto, ST - 1):
                    s_dma(next_s[0])
                    next_s[0] += 1

            s_prep(0, scale_on_dve=True)
            s_prep(1, scale_on_dve=True)
            q_prep(0)
            s_prep(2, scale_on_dve=True)
            s_dma_ahead(5)
            s_prep(3, scale_on_dve=True)

            # s-prep schedule: pass w preps tiles 4w+4 .. 4w+7 (w<4)
            mxs = [None] * MT
            prev = [None, None]
            for w in range(WAY):
                for m in range(MT):
                    if w == 0:
                        if m + 2 < MT:
                            q_dma(m + 2)
                        if m + 1 < MT:
                            q_prep(m + 1)
                    if w < 4 and m in (1, 5, 9, 13):
                        t = 4 * (w + 1) + (m - 1) // 4
                        s_dma_ahead(t + 2)
                        s_prep(t)
                    if w == 0:
                        mxs[m] = mxp.tile([P, WAY * 8], f32, tag="mx",
                                          name=f"mx{m}")
                    psc = spp.tile([P, NW], f32, tag="psc",
                                   name=f"psc{m}_{w}")
                    for c in range(KC):
                        nc.tensor.matmul(
                            psc,
                            qT(c)[:, m * P:(m + 1) * P],
                            sT(c)[:, w * NW:(w + 1) * NW],
                            start=(c == 0),
                            stop=(c == KC - 1),
                        )
                    nc.vector.max(mxs[m][:, w * 8:(w + 1) * 8], psc)
                    if w == WAY - 1:
                        tsum = small.tile([P, WAY], f32, tag="tsum")
                        nc.vector.tensor_reduce(
                            tsum,
                            mxs[m].rearrange("p (w j) -> p w j", w=WAY)[:, :, :k],
                            axis=mybir.AxisListType.X,
                            op=mybir.AluOpType.add,
                        )
                        scaled = small.tile([P, WAY], f32, tag="scaled")
                        nc.scalar.mul(scaled, tsum, qinv[:, m:m + 1])
                        if prev[0] is not None:
                            nc.tensor.matmul(
                                out_ps,
                                ind_sb[:, prev[1] * NQ:(prev[1] + 1) * NQ],
                                prev[0], start=(prev[1] == 0), stop=False)
                        prev = [scaled, m]
            nc.tensor.matmul(
                out_ps, ind_sb[:, prev[1] * NQ:(prev[1] + 1) * NQ],
                prev[0], start=False, stop=True)
            out_sb = small.tile([NQ, WAY], f32, tag="out_sb")
            nc.scalar.copy(out_sb, out_ps)
            nc.sync.dma_start(out=out_d, in_=out_sb)

    nc.compile()
    return nc


def get_program(k: int):
    if k not in _prog_cache:
        _prog_cache[k] = _build(k)
    return _prog_cache[k]


def make_in_maps(input1: np.ndarray, input2: np.ndarray):
    """Shard full inputs into per-core input maps."""
    input1 = np.ascontiguousarray(np.asarray(input1), dtype=np.float32)
    input2 = np.ascontiguousarray(np.asarray(input2), dtype=np.float32)
    in_maps = []
    for core in range(N_CORES):
        b = core // 4
        qs = (core % 4) * NQ
        qe = min(Q, qs + NQ)
        nq = qe - qs
        qdat = input1[b].reshape(Q, HW, C)[qs:qe].reshape(-1, C)
        qfull = np.ones((PAD_P, C), np.float32)
        qfull[: nq * HW] = qdat
        sfull = np.ones((PAD_S, C), np.float32)
        sfull[:NS] = input2[b].reshape(NS, C)
        # indicator: patch row p of M-tile t belongs to query (t*128+p)//HW
        ind = np.zeros((P, MT * NQ), np.float32)
        g = np.arange(MT * P)
        j = g // HW
        valid = j < nq
        ind[g[valid] % P, (g[valid] // P) * NQ + j[valid]] = 1.0
        in_maps.append({"q": qfull, "s": sfull, "ind": ind,
                        "ident": np.eye(P, dtype=np.float32)})
    return in_maps


def gather_out(results) -> np.ndarray:
    out = np.zeros((B, Q, WAY), np.float32)
    for core in range(N_CORES):
        b = core // 4
        qs = (core % 4) * NQ
        n = min(Q, qs + NQ) - qs
        out[b, qs:qs + n] = results[core]["out"][:n]
    return out


def kernel(input1, input2, neighbor_k):
    k = int(np.asarray(neighbor_k))
    nc = get_program(k)
    in_maps = make_in_maps(input1, input2)
    res = run_bass_kernel_spmd(nc, in_maps, core_ids=list(range(N_CORES)))
    return gather_out(res.results)


# revision 29
# speedup vs baseline: 11528.4078x; 10172.2814x over previous
"""TRN2 Bass kernel for nn_MetaBaseline (DN4-style local-descriptor kNN).

Reference computation (per batch b):
  q = normalize(input1[b].reshape(75, 100, 640), axis=-1)      # query patches
  s = normalize(input2[b].reshape(2500, 640), axis=-1)         # support descs
  scores = q_patches @ s.T                                     # [7500, 2500]
  per way group g (columns [500g, 500g+500)): top-k per row, mean,
  then sum over the 100 patches of each query -> out [75, 5].

Sharding: data-parallel over (b, query-quarter): 8 cores, each handles one
batch's quarter of queries (19 queries padded) with that batch's full
support replicated.

Per-core device program. Engines execute in emission order, so emission is
software-pipelined. The score loop is WAY-OUTER: pass w only needs support
descriptor tiles 0..4w+3, so score matmuls start as soon as the first four
support tiles are normalized+transposed; the remaining support prep streams
in the background during passes 0-3, and query prep (norm chain, packed PE
transposes, float32r eviction) is folded into pass 0 one tile ahead.
Top-8 per (patch, way) via DVE max straight from the PSUM score bank
(bank freed immediately after); pass 4 finishes each patch tile with a
strided top-k tensor_reduce, ACT scale by 1/(k*|q_patch|), and a small
fp32 indicator matmul accumulating per-query sums in PSUM -> [19, 5].
"""
import os
from contextlib import ExitStack

import numpy as np

import concourse.bass as bass  # noqa: F401
import concourse.mybir as mybir
import concourse.tile as tile
from concourse import bacc
from concourse.bass_utils import run_bass_kernel_spmd

# Problem geometry (hardcoded per contest rules)
B, Q, WAY, SHOT, H, W, C = 2, 75, 5, 5, 10, 10, 640
HW = H * W               # 100 patches per query / support image
NQ = 19                  # queries per core (4 cores x 19 = 76 >= 75)
MT = 15                  # patch M-tiles of 128 -> 1920 rows (1900 real)
PAD_P = MT * 128
NS = WAY * SHOT * HW     # 2500 support descriptors per batch
ST = 20                  # support tiles of 128 -> 2560 rows
PAD_S = ST * 128
KC = 5                   # C chunks of 128 (640 = 5*128)
P = 128
NW = SHOT * HW           # 500 support descriptors per way group
N_CORES = 8
N_WARM = int(os.environ.get("N_WARM", "14"))

_prog_cache: dict[int, object] = {}


def _build(k: int):
    """Build + compile the per-core SPMD program for neighbor_k == k."""
    assert 1 <= k <= 8, f"neighbor_k={k} not supported (need 1..8)"
    nc = bacc.Bacc("TRN2", target_bir_lowering=False, debug=False)
    f32 = mybir.dt.float32
    f32r = mybir.dt.float32r
    AF = mybir.ActivationFunctionType

    q_d = nc.dram_tensor("q", [PAD_P, C], f32, kind="ExternalInput").ap()
    s_d = nc.dram_tensor("s", [PAD_S, C], f32, kind="ExternalInput").ap()
    ind_d = nc.dram_tensor("ind", [P, MT * NQ], f32, kind="ExternalInput").ap()
    ident_d = nc.dram_tensor("ident", [P, P], f32, kind="ExternalInput").ap()
    out_d = nc.dram_tensor("out", [NQ, WAY], f32, kind="ExternalOutput").ap()

    with tile.TileContext(nc) as tc:
        with ExitStack() as ctx:
            const = ctx.enter_context(tc.tile_pool(name="const", bufs=1))
            big = ctx.enter_context(tc.tile_pool(name="big", bufs=1))
            loads = ctx.enter_context(tc.tile_pool(name="loads", bufs=6))
            small = ctx.enter_context(tc.tile_pool(name="small", bufs=4))
            mxp = ctx.enter_context(tc.tile_pool(name="mxp", bufs=MT))
            outp = ctx.enter_context(
                tc.tile_pool(name="outp", bufs=1, space="PSUM")
            )
            tp4 = ctx.enter_context(
                tc.tile_pool(name="tp4", bufs=2, space="PSUM")
            )
            tp1 = ctx.enter_context(
                tc.tile_pool(name="tp1", bufs=2, space="PSUM")
            )
            spp = ctx.enter_context(
                tc.tile_pool(name="spp", bufs=3, space="PSUM")
            )

            ident = const.tile([P, P], f32)
            ident_r = const.tile([P, P], f32r, name="ident_r")
            ind_sb = const.tile([P, MT * NQ], f32)
            qinv = const.tile([P, MT], f32)

            # chunk c of each transposed tensor has its own column band so a
            # packed 4-chunk PSUM bank evicts with one strided copy
            s_T = big.tile([P, KC * PAD_S], f32r, name="s_T")
            q_T = big.tile([P, KC * PAD_P], f32r, name="q_T")

            def sT(c):
                return s_T[:, c * PAD_S:(c + 1) * PAD_S]

            def qT(c):
                return q_T[:, c * PAD_P:(c + 1) * PAD_P]

            out_ps = outp.tile([NQ, WAY], f32)

            # ---- warmups: ACT tables + PE pipeline (no DMA deps) ----
            wtile = const.tile([P, P], f32, name="wtile")
            nc.gpsimd.memset(wtile, 1.0)
            wsq = small.tile([P, 1], f32, tag="snrm")
            nc.scalar.sqrt(wsq, wtile[:, 0:1])
            wps = tp4.tile([P, 4 * P], f32, tag="tp4")
            for i in range(N_WARM):
                nc.tensor.transpose(
                    wps[:, (i % 4) * P:(i % 4 + 1) * P], wtile, wtile
                )

            nev = 0

            def evict(out_ap, src_ap):
                nonlocal nev
                if nev % 2 == 0:
                    nc.vector.tensor_copy(out_ap, src_ap)
                else:
                    nc.scalar.copy(out_ap, src_ap)
                nev += 1

            def transpose_evict(x, T_all, T_pad, t, defer=False):
                """5 packed PE transposes of x into T_all's column bands."""
                isr = x.dtype == f32r
                idn = ident_r if isr else ident
                psA = tp4.tile([P, 4 * P], f32, tag="tp4", name=f"psA_{t}")
                psB = tp1.tile([P, P], f32, tag="tp1", name=f"psB_{t}")
                psAv = psA.bitcast(f32r) if isr else psA
                psBv = psB.bitcast(f32r) if isr else psB
                for c in range(4):
                    nc.tensor.transpose(
                        psAv[:, c * P:(c + 1) * P],
                        x[:, c * P:(c + 1) * P], idn)
                nc.tensor.transpose(psBv, x[:, 4 * P:5 * P], idn)
                out_ap = T_all[:, :4 * T_pad].rearrange(
                    "p (c n) -> p c n", c=4
                )[:, :, t * P:(t + 1) * P]

                def _ev():
                    evict(out_ap, psA.rearrange("p (c n) -> p c n", c=4))
                    evict(
                        T_all[:, 4 * T_pad + t * P:4 * T_pad + (t + 1) * P],
                        psB)
                if defer:
                    return _ev
                _ev()

            xs_s = [None] * ST
            xs_q = [None] * MT

            def s_dma(t, split=1):
                x = loads.tile([P, C], f32, tag="x_tile", name=f"sx{t}")
                h = P // split
                for i in range(split):
                    nc.sync.dma_start(
                        out=x[i * h:(i + 1) * h, :],
                        in_=s_d[t * P + i * h:t * P + (i + 1) * h, :])
                xs_s[t] = x

            def q_dma(m, split=1):
                x = loads.tile([P, C], f32, tag="x_tile", name=f"qx{m}")
                h = P // split
                for i in range(split):
                    nc.sync.dma_start(
                        out=x[i * h:(i + 1) * h, :],
                        in_=q_d[m * P + i * h:m * P + (i + 1) * h, :])
                xs_q[m] = x

            def s_prep(t, sq_on_dve=False, scale_on_dve=False,
                       defer=False):
                x = xs_s[t]
                sq = loads.tile([P, C], f32, tag="sq", name=f"ssq{t}")
                ssum = small.tile([P, 1], f32, tag="ssum")
                if sq_on_dve:
                    nc.vector.tensor_tensor_reduce(
                        sq, x, x, 1.0, 0.0,
                        mybir.AluOpType.mult, mybir.AluOpType.add, ssum)
                else:
                    nc.scalar.activation(sq, x, AF.Square, accum_out=ssum)
                snrm = small.tile([P, 1], f32, tag="snrm")
                nc.scalar.sqrt(snrm, ssum)
                sinv = small.tile([P, 1], f32, tag="sinv")
                nc.vector.reciprocal(sinv, snrm)
                s_n = loads.tile([P, C], f32r, tag="s_n", name=f"sn{t}")
                if scale_on_dve:
                    nc.vector.tensor_scalar_mul(s_n, x, sinv)
                else:
                    # NOTE: never gpsimd here - tensor_scalar on GPSIMD
                    # measures ~9.3us per [128,640] tile on real TRN2
                    nc.scalar.mul(s_n, x, sinv)
                return transpose_evict(s_n, s_T, PAD_S, t, defer=defer)

            def q_prep(m, defer=False):
                x = xs_q[m]
                ev = transpose_evict(x, q_T, PAD_P, m, defer=defer)
                sq = loads.tile([P, C], f32, tag="sq", name=f"qsq{m}")
                qsum = small.tile([P, 1], f32, tag="ssum")
                nc.scalar.activation(sq, x, AF.Square, accum_out=qsum)
                kn = small.tile([P, 1], f32, tag="snrm")
                # sqrt(k^2 * sum(q^2)) = k * |q|
                nc.scalar.activation(kn, qsum, AF.Sqrt, scale=float(k * k))
                nc.vector.reciprocal(qinv[:, m:m + 1], kn)
                return ev

            # ---- prologue: support tiles 0-3, queries 0-1 ----
            # DMA order: support first (its prep chain is the pace-setter),
            # then ident (first needed by real transposes), queries, ind.
            for t in range(4):
                s_dma(t)
            nc.sync.dma_start(out=ident, in_=ident_d)
            nc.vector.tensor_copy(ident_r, ident)
            q_dma(0)
            q_dma(1)
            nc.sync.dma_start(out=ind_sb, in_=ind_d)
            next_s = [4]

            def s_dma_ahead(upto):
                while next_s[0] <= min(upto, ST - 1):
                    s_dma(next_s[0])
                    next_s[0] += 1

            s_prep(0, scale_on_dve=True)
            s_prep(1, scale_on_dve=True)
            q_prep(0)
            s_prep(2, scale_on_dve=True)
            s_dma_ahead(5)
            s_prep(3, scale_on_dve=True)

            # s-prep schedule: pass w preps tiles 4w+4 .. 4w+7 (w<4)
            mxs = [None] * MT
            prev = [None, None]
            for w in range(WAY):
                for m in range(MT):
                    if w == 0:
                        if m + 2 < MT:
                            q_dma(m + 2)
                        if m + 1 < MT:
                            q_prep(m + 1)
                    if w < 4 and m in (1, 5, 9, 13):
                        t = 4 * (w + 1) + (m - 1) // 4
                        s_dma_ahead(t + 2)
                        s_prep(t, scale_on_dve=(t % 2 == 1))
                    if w == 0:
                        mxs[m] = mxp.tile([P, WAY * 8], f32, tag="mx",
                                          name=f"mx{m}")
                    psc = spp.tile([P, NW], f32, tag="psc",
                                   name=f"psc{m}_{w}")
                    for c in range(KC):
                        nc.tensor.matmul(
                            psc,
                            qT(c)[:, m * P:(m + 1) * P],
                            sT(c)[:, w * NW:(w + 1) * NW],
                            start=(c == 0),
                            stop=(c == KC - 1),
                        )
                    nc.vector.max(mxs[m][:, w * 8:(w + 1) * 8], psc)
                    if w == WAY - 1:
                        tsum = small.tile([P, WAY], f32, tag="tsum")
                        nc.vector.tensor_reduce(
                            tsum,
                            mxs[m].rearrange("p (w j) -> p w j", w=WAY)[:, :, :k],
                            axis=mybir.AxisListType.X,
                            op=mybir.AluOpType.add,
                        )
                        scaled = small.tile([P, WAY], f32, tag="scaled")
                        nc.scalar.mul(scaled, tsum, qinv[:, m:m + 1])
                        if prev[0] is not None:
                            nc.tensor.matmul(
                                out_ps,
                                ind_sb[:, prev[1] * NQ:(prev[1] + 1) * NQ],
                                prev[0], start=(prev[1] == 0), stop=False)
                        prev = [scaled, m]
            nc.tensor.matmul(
                out_ps, ind_sb[:, prev[1] * NQ:(prev[1] + 1) * NQ],
                prev[0], start=False, stop=True)
            out_sb = small.tile([NQ, WAY], f32, tag="out_sb")
            nc.scalar.copy(out_sb, out_ps)
            nc.sync.dma_start(out=out_d, in_=out_sb)

    nc.compile()
    return nc


def get_program(k: int):
    if k not in _prog_cache:
        _prog_cache[k] = _build(k)
    return _prog_cache[k]


def make_in_maps(input1: np.ndarray, input2: np.ndarray):
    """Shard full inputs into per-core input maps."""
    input1 = np.ascontiguousarray(np.asarray(input1), dtype=np.float32)
    input2 = np.ascontiguousarray(np.asarray(input2), dtype=np.float32)
    in_maps = []
    for core in range(N_CORES):
        b = core // 4
        qs = (core % 4) * NQ
        qe = min(Q, qs + NQ)
        nq = qe - qs
        qdat = input1[b].reshape(Q, HW, C)[qs:qe].reshape(-1, C)
        qfull = np.ones((PAD_P, C), np.float32)
        qfull[: nq * HW] = qdat
        sfull = np.ones((PAD_S, C), np.float32)
        sfull[:NS] = input2[b].reshape(NS, C)
        # indicator: patch row p of M-tile t belongs to query (t*128+p)//HW
        ind = np.zeros((P, MT * NQ), np.float32)
        g = np.arange(MT * P)
        j = g // HW
        valid = j < nq
        ind[g[valid] % P, (g[valid] // P) * NQ + j[valid]] = 1.0
        in_maps.append({"q": qfull, "s": sfull, "ind": ind,
                        "ident": np.eye(P, dtype=np.float32)})
    return in_maps


def gather_out(results) -> np.ndarray:
    out = np.zeros((B, Q, WAY), np.float32)
    for core in range(N_CORES):
        b = core // 4
        qs = (core % 4) * NQ
        n = min(Q, qs + NQ) - qs
        out[b, qs:qs + n] = results[core]["out"][:n]
    return out


def kernel(input1, input2, neighbor_k):
    k = int(np.asarray(neighbor_k))
    nc = get_program(k)
    in_maps = make_in_maps(input1, input2)
    # the axon-tunneled device occasionally reports a transient
    # "unrecoverable" state right after a previous process's teardown;
    # it recovers within seconds, so retry a couple of times
    import time
    last = None
    for attempt in range(3):
        try:
            res = run_bass_kernel_spmd(
                nc, in_maps, core_ids=list(range(N_CORES)))
            return gather_out(res.results)
        except Exception as e:  # noqa: BLE001
            last = e
            if attempt < 2:
                time.sleep(20.0 * (attempt + 1))
    raise last


# revision 30
# speedup vs baseline: 11855.5294x; 1.0284x over previous
"""TRN2 Bass kernel for nn_MetaBaseline (DN4-style local-descriptor kNN).

Reference computation (per batch b):
  q = normalize(input1[b].reshape(75, 100, 640), axis=-1)      # query patches
  s = normalize(input2[b].reshape(2500, 640), axis=-1)         # support descs
  scores = q_patches @ s.T                                     # [7500, 2500]
  per way group g (columns [500g, 500g+500)): top-k per row, mean,
  then sum over the 100 patches of each query -> out [75, 5].

Sharding: data-parallel over (b, query-quarter): 8 cores, each handles one
batch's quarter of queries (19 queries padded) with that batch's full
support replicated.

Per-core device program. Engines execute in emission order, so emission is
software-pipelined. The score loop is WAY-OUTER: pass w only needs support
descriptor tiles 0..4w+3, so score matmuls start as soon as the first four
support tiles are normalized+transposed; the remaining support prep streams
in the background during passes 0-3, and query prep (norm chain, packed PE
transposes, float32r eviction) is folded into pass 0 one tile ahead.
Top-8 per (patch, way) via DVE max straight from the PSUM score bank
(bank freed immediately after); pass 4 finishes each patch tile with a
strided top-k tensor_reduce, ACT scale by 1/(k*|q_patch|), and a small
fp32 indicator matmul accumulating per-query sums in PSUM -> [19, 5].
"""
import os
from contextlib import ExitStack

import numpy as np

import concourse.bass as bass  # noqa: F401
import concourse.mybir as mybir
import concourse.tile as tile
from concourse import bacc
from concourse.bass_utils import run_bass_kernel_spmd

# Problem geometry (hardcoded per contest rules)
B, Q, WAY, SHOT, H, W, C = 2, 75, 5, 5, 10, 10, 640
HW = H * W               # 100 patches per query / support image
NQ = 19                  # queries per core (4 cores x 19 = 76 >= 75)
MT = 15                  # patch M-tiles of 128 -> 1920 rows (1900 real)
PAD_P = MT * 128
NS = WAY * SHOT * HW     # 2500 support descriptors per batch
ST = 20                  # support tiles of 128 -> 2560 rows
PAD_S = ST * 128
KC = 5                   # C chunks of 128 (640 = 5*128)
P = 128
NW = SHOT * HW           # 500 support descriptors per way group
N_CORES = 8
N_WARM = int(os.environ.get("N_WARM", "14"))
BF16 = os.environ.get("BF16", "0") == "1"  # experimental: bf16 score operands

_prog_cache: dict[int, object] = {}


def _build(k: int):
    """Build + compile the per-core SPMD program for neighbor_k == k."""
    assert 1 <= k <= 8, f"neighbor_k={k} not supported (need 1..8)"
    nc = bacc.Bacc("TRN2", target_bir_lowering=False, debug=False)
    f32 = mybir.dt.float32
    f32r = mybir.dt.float32r
    t_dt = mybir.dt.bfloat16 if BF16 else f32r
    AF = mybir.ActivationFunctionType

    q_d = nc.dram_tensor("q", [PAD_P, C], f32, kind="ExternalInput").ap()
    s_d = nc.dram_tensor("s", [PAD_S, C], f32, kind="ExternalInput").ap()
    ind_d = nc.dram_tensor("ind", [P, MT * NQ], f32, kind="ExternalInput").ap()
    ident_d = nc.dram_tensor("ident", [P, P], f32, kind="ExternalInput").ap()
    out_d = nc.dram_tensor("out", [NQ, WAY], f32, kind="ExternalOutput").ap()

    with tile.TileContext(nc) as tc:
        with ExitStack() as ctx:
            const = ctx.enter_context(tc.tile_pool(name="const", bufs=1))
            big = ctx.enter_context(tc.tile_pool(name="big", bufs=1))
            loads = ctx.enter_context(tc.tile_pool(name="loads", bufs=6))
            small = ctx.enter_context(tc.tile_pool(name="small", bufs=4))
            mxp = ctx.enter_context(tc.tile_pool(name="mxp", bufs=MT))
            outp = ctx.enter_context(
                tc.tile_pool(name="outp", bufs=1, space="PSUM")
            )
            tp4 = ctx.enter_context(
                tc.tile_pool(name="tp4", bufs=2, space="PSUM")
            )
            tp1 = ctx.enter_context(
                tc.tile_pool(name="tp1", bufs=2, space="PSUM")
            )
            spp = ctx.enter_context(
                tc.tile_pool(name="spp", bufs=3, space="PSUM")
            )

            ident = const.tile([P, P], f32)
            ident_r = const.tile([P, P], f32r, name="ident_r")
            ind_sb = const.tile([P, MT * NQ], f32)
            qinv = const.tile([P, MT], f32)

            # chunk c of each transposed tensor has its own column band so a
            # packed 4-chunk PSUM bank evicts with one strided copy
            s_T = big.tile([P, KC * PAD_S], t_dt, name="s_T")
            q_T = big.tile([P, KC * PAD_P], t_dt, name="q_T")

            def sT(c):
                return s_T[:, c * PAD_S:(c + 1) * PAD_S]

            def qT(c):
                return q_T[:, c * PAD_P:(c + 1) * PAD_P]

            out_ps = outp.tile([NQ, WAY], f32)

            # ---- warmups: ACT tables + PE pipeline (no DMA deps) ----
            wtile = const.tile([P, P], f32, name="wtile")
            nc.gpsimd.memset(wtile, 1.0)
            wsq = small.tile([P, 1], f32, tag="snrm")
            nc.scalar.sqrt(wsq, wtile[:, 0:1])
            wps = tp4.tile([P, 4 * P], f32, tag="tp4")
            for i in range(N_WARM):
                nc.tensor.transpose(
                    wps[:, (i % 4) * P:(i % 4 + 1) * P], wtile, wtile
                )

            nev = 0

            def evict(out_ap, src_ap):
                nonlocal nev
                if nev % 2 == 0:
                    nc.vector.tensor_copy(out_ap, src_ap)
                else:
                    nc.scalar.copy(out_ap, src_ap)
                nev += 1

            def transpose_evict(x, T_all, T_pad, t, defer=False):
                """5 packed PE transposes of x into T_all's column bands."""
                isr = x.dtype == f32r
                idn = ident_r if isr else ident
                psA = tp4.tile([P, 4 * P], f32, tag="tp4", name=f"psA_{t}")
                psB = tp1.tile([P, P], f32, tag="tp1", name=f"psB_{t}")
                psAv = psA.bitcast(f32r) if isr else psA
                psBv = psB.bitcast(f32r) if isr else psB
                for c in range(4):
                    nc.tensor.transpose(
                        psAv[:, c * P:(c + 1) * P],
                        x[:, c * P:(c + 1) * P], idn)
                nc.tensor.transpose(psBv, x[:, 4 * P:5 * P], idn)
                out_ap = T_all[:, :4 * T_pad].rearrange(
                    "p (c n) -> p c n", c=4
                )[:, :, t * P:(t + 1) * P]

                def _ev():
                    evict(out_ap, psA.rearrange("p (c n) -> p c n", c=4))
                    evict(
                        T_all[:, 4 * T_pad + t * P:4 * T_pad + (t + 1) * P],
                        psB)
                if defer:
                    return _ev
                _ev()

            xs_s = [None] * ST
            xs_q = [None] * MT

            def s_dma(t, split=1):
                x = loads.tile([P, C], f32, tag="x_tile", name=f"sx{t}")
                h = P // split
                for i in range(split):
                    nc.sync.dma_start(
                        out=x[i * h:(i + 1) * h, :],
                        in_=s_d[t * P + i * h:t * P + (i + 1) * h, :])
                xs_s[t] = x

            def q_dma(m, split=1):
                x = loads.tile([P, C], f32, tag="x_tile", name=f"qx{m}")
                h = P // split
                for i in range(split):
                    nc.sync.dma_start(
                        out=x[i * h:(i + 1) * h, :],
                        in_=q_d[m * P + i * h:m * P + (i + 1) * h, :])
                xs_q[m] = x

            def s_prep(t, sq_on_dve=False, scale_on_dve=False,
                       defer=False):
                x = xs_s[t]
                sq = loads.tile([P, C], f32, tag="sq", name=f"ssq{t}")
                ssum = small.tile([P, 1], f32, tag="ssum")
                if sq_on_dve:
                    nc.vector.tensor_tensor_reduce(
                        sq, x, x, 1.0, 0.0,
                        mybir.AluOpType.mult, mybir.AluOpType.add, ssum)
                else:
                    nc.scalar.activation(sq, x, AF.Square, accum_out=ssum)
                snrm = small.tile([P, 1], f32, tag="snrm")
                nc.scalar.sqrt(snrm, ssum)
                sinv = small.tile([P, 1], f32, tag="sinv")
                nc.vector.reciprocal(sinv, snrm)
                s_n = loads.tile([P, C], f32r, tag="s_n", name=f"sn{t}")
                if scale_on_dve:
                    nc.vector.tensor_scalar_mul(s_n, x, sinv)
                else:
                    # NOTE: never gpsimd here - tensor_scalar on GPSIMD
                    # measures ~9.3us per [128,640] tile on real TRN2
                    nc.scalar.mul(s_n, x, sinv)
                return transpose_evict(s_n, s_T, PAD_S, t, defer=defer)

            def q_prep(m, defer=False):
                x = xs_q[m]
                ev = transpose_evict(x, q_T, PAD_P, m, defer=defer)
                sq = loads.tile([P, C], f32, tag="sq", name=f"qsq{m}")
                qsum = small.tile([P, 1], f32, tag="ssum")
                nc.scalar.activation(sq, x, AF.Square, accum_out=qsum)
                kn = small.tile([P, 1], f32, tag="snrm")
                # sqrt(k^2 * sum(q^2)) = k * |q|
                nc.scalar.activation(kn, qsum, AF.Sqrt, scale=float(k * k))
                nc.vector.reciprocal(qinv[:, m:m + 1], kn)
                return ev

            # ---- prologue: support tiles 0-3, queries 0-1 ----
            # DMA order: support first (its prep chain is the pace-setter),
            # then ident (first needed by real transposes), queries, ind.
            for t in range(4):
                s_dma(t)
            nc.sync.dma_start(out=ident, in_=ident_d)
            nc.vector.tensor_copy(ident_r, ident)
            q_dma(0)
            q_dma(1)
            nc.sync.dma_start(out=ind_sb, in_=ind_d)
            next_s = [4]

            def s_dma_ahead(upto):
                while next_s[0] <= min(upto, ST - 1):
                    s_dma(next_s[0])
                    next_s[0] += 1

            s_prep(0, scale_on_dve=True)
            s_prep(1, scale_on_dve=True)
            q_prep(0)
            s_prep(2, scale_on_dve=True)
            s_dma_ahead(5)
            s_prep(3, scale_on_dve=True)

            # s-prep schedule: pass w preps tiles 4w+4 .. 4w+7 (w<4)
            mxs = [None] * MT
            prev = [None, None]
            for w in range(WAY):
                for m in range(MT):
                    if w == 0:
                        if m + 2 < MT:
                            q_dma(m + 2)
                        if m + 1 < MT:
                            q_prep(m + 1)
                    if w < 4 and m in (1, 5, 9, 13):
                        t = 4 * (w + 1) + (m - 1) // 4
                        s_dma_ahead(t + 2)
                        s_prep(t, scale_on_dve=(t % 2 == 1))
                    if w == 0:
                        mxs[m] = mxp.tile([P, WAY * 8], f32, tag="mx",
                                          name=f"mx{m}")
                    psc = spp.tile([P, NW], f32, tag="psc",
                                   name=f"psc{m}_{w}")
                    for c in range(KC):
                        nc.tensor.matmul(
                            psc,
                            qT(c)[:, m * P:(m + 1) * P],
                            sT(c)[:, w * NW:(w + 1) * NW],
                            start=(c == 0),
                            stop=(c == KC - 1),
                        )
                    nc.vector.max(mxs[m][:, w * 8:(w + 1) * 8], psc)
                    if w == WAY - 1:
                        tsum = small.tile([P, WAY], f32, tag="tsum")
                        nc.vector.tensor_reduce(
                            tsum,
                            mxs[m].rearrange("p (w j) -> p w j", w=WAY)[:, :, :k],
                            axis=mybir.AxisListType.X,
                            op=mybir.AluOpType.add,
                        )
                        scaled = small.tile([P, WAY], f32, tag="scaled")
                        nc.scalar.mul(scaled, tsum, qinv[:, m:m + 1])
                        if prev[0] is not None:
                            nc.tensor.matmul(
                                out_ps,
                                ind_sb[:, prev[1] * NQ:(prev[1] + 1) * NQ],
                                prev[0], start=(prev[1] == 0), stop=False)
                        prev = [scaled, m]
            nc.tensor.matmul(
                out_ps, ind_sb[:, prev[1] * NQ:(prev[1] + 1) * NQ],
                prev[0], start=False, stop=True)
            out_sb = small.tile([NQ, WAY], f32, tag="out_sb")
            nc.scalar.copy(out_sb, out_ps)
            nc.sync.dma_start(out=out_d, in_=out_sb)

    nc.compile()
    return nc


def get_program(k: int):
    if k not in _prog_cache:
        _prog_cache[k] = _build(k)
    return _prog_cache[k]


def make_in_maps(input1: np.ndarray, input2: np.ndarray):
    """Shard full inputs into per-core input maps."""
    input1 = np.ascontiguousarray(np.asarray(input1), dtype=np.float32)
    input2 = np.ascontiguousarray(np.asarray(input2), dtype=np.float32)
    in_maps = []
    for core in range(N_CORES):
        b = core // 4
        qs = (core % 4) * NQ
        qe = min(Q, qs + NQ)
        nq = qe - qs
        qdat = input1[b].reshape(Q, HW, C)[qs:qe].reshape(-1, C)
        qfull = np.ones((PAD_P, C), np.float32)
        qfull[: nq * HW] = qdat
        sfull = np.ones((PAD_S, C), np.float32)
        sfull[:NS] = input2[b].reshape(NS, C)
        # indicator: patch row p of M-tile t belongs to query (t*128+p)//HW
        ind = np.zeros((P, MT * NQ), np.float32)
        g = np.arange(MT * P)
        j = g // HW
        valid = j < nq
        ind[g[valid] % P, (g[valid] // P) * NQ + j[valid]] = 1.0
        in_maps.append({"q": qfull, "s": sfull, "ind": ind,
                        "ident": np.eye(P, dtype=np.float32)})
    return in_maps


def gather_out(results) -> np.ndarray:
    out = np.zeros((B, Q, WAY), np.float32)
    for core in range(N_CORES):
        b = core // 4
        qs = (core % 4) * NQ
        n = min(Q, qs + NQ) - qs
        out[b, qs:qs + n] = results[core]["out"][:n]
    return out


def kernel(input1, input2, neighbor_k):
    k = int(np.asarray(neighbor_k))
    nc = get_program(k)
    in_maps = make_in_maps(input1, input2)
    # the axon-tunneled device occasionally reports a transient
    # "unrecoverable" state right after a previous process's teardown;
    # it recovers within seconds, so retry a couple of times
    import time
    last = None
    for attempt in range(3):
        try:
            res = run_bass_kernel_spmd(
                nc, in_maps, core_ids=list(range(N_CORES)))
            return gather_out(res.results)
        except Exception as e:  # noqa: BLE001
            last = e
            if attempt < 2:
                time.sleep(20.0 * (attempt + 1))
    raise last


# revision 31
# speedup vs baseline: 11865.6707x; 1.0009x over previous
"""TRN2 Bass kernel for nn_MetaBaseline (DN4-style local-descriptor kNN).

Reference computation (per batch b):
  q = normalize(input1[b].reshape(75, 100, 640), axis=-1)      # query patches
  s = normalize(input2[b].reshape(2500, 640), axis=-1)         # support descs
  scores = q_patches @ s.T                                     # [7500, 2500]
  per way group g (columns [500g, 500g+500)): top-k per row, mean,
  then sum over the 100 patches of each query -> out [75, 5].

Sharding: data-parallel over (b, query-quarter): 8 cores, each handles one
batch's quarter of queries (19 queries padded) with that batch's full
support replicated.

Per-core device program. Engines execute in emission order, so emission is
software-pipelined. The score loop is WAY-OUTER: pass w only needs support
descriptor tiles 0..4w+3, so score matmuls start as soon as the first four
support tiles are normalized+transposed; the remaining support prep streams
in the background during passes 0-3, and query prep (norm chain, packed PE
transposes, float32r eviction) is folded into pass 0 one tile ahead.
Top-8 per (patch, way) via DVE max straight from the PSUM score bank
(bank freed immediately after); pass 4 finishes each patch tile with a
strided top-k tensor_reduce, ACT scale by 1/(k*|q_patch|), and a small
fp32 indicator matmul accumulating per-query sums in PSUM -> [19, 5].
"""
import os
from contextlib import ExitStack

import numpy as np

import concourse.bass as bass  # noqa: F401
import concourse.mybir as mybir
import concourse.tile as tile
from concourse import bacc
from concourse.bass_utils import run_bass_kernel_spmd

# Problem geometry (hardcoded per contest rules)
B, Q, WAY, SHOT, H, W, C = 2, 75, 5, 5, 10, 10, 640
HW = H * W               # 100 patches per query / support image
NQ = 19                  # queries per core (4 cores x 19 = 76 >= 75)
MT = 15                  # patch M-tiles of 128 -> 1920 rows (1900 real)
PAD_P = MT * 128
NS = WAY * SHOT * HW     # 2500 support descriptors per batch
ST = 20                  # support tiles of 128 -> 2560 rows
PAD_S = ST * 128
KC = 5                   # C chunks of 128 (640 = 5*128)
P = 128
NW = SHOT * HW           # 500 support descriptors per way group
N_CORES = 8
N_WARM = int(os.environ.get("N_WARM", "14"))
BF16 = os.environ.get("BF16", "0") == "1"  # experimental: bf16 score operands

_prog_cache: dict[int, object] = {}


def _build(k: int):
    """Build + compile the per-core SPMD program for neighbor_k == k."""
    assert 1 <= k <= 8, f"neighbor_k={k} not supported (need 1..8)"
    nc = bacc.Bacc("TRN2", target_bir_lowering=False, debug=False)
    f32 = mybir.dt.float32
    f32r = mybir.dt.float32r
    t_dt = mybir.dt.bfloat16 if BF16 else f32r
    AF = mybir.ActivationFunctionType

    q_d = nc.dram_tensor("q", [PAD_P, C], f32, kind="ExternalInput").ap()
    s_d = nc.dram_tensor("s", [PAD_S, C], f32, kind="ExternalInput").ap()
    ind_d = nc.dram_tensor("ind", [P, MT * NQ], f32, kind="ExternalInput").ap()
    ident_d = nc.dram_tensor("ident", [P, P], f32, kind="ExternalInput").ap()
    out_d = nc.dram_tensor("out", [NQ, WAY], f32, kind="ExternalOutput").ap()

    with tile.TileContext(nc) as tc:
        with ExitStack() as ctx:
            const = ctx.enter_context(tc.tile_pool(name="const", bufs=1))
            big = ctx.enter_context(tc.tile_pool(name="big", bufs=1))
            loads = ctx.enter_context(tc.tile_pool(name="loads", bufs=6))
            small = ctx.enter_context(tc.tile_pool(name="small", bufs=4))
            mxp = ctx.enter_context(tc.tile_pool(name="mxp", bufs=MT))
            outp = ctx.enter_context(
                tc.tile_pool(name="outp", bufs=1, space="PSUM")
            )
            tp4 = ctx.enter_context(
                tc.tile_pool(name="tp4", bufs=2, space="PSUM")
            )
            tp1 = ctx.enter_context(
                tc.tile_pool(name="tp1", bufs=1, space="PSUM")
            )
            spp = ctx.enter_context(
                tc.tile_pool(name="spp", bufs=4, space="PSUM")
            )

            ident = const.tile([P, P], f32)
            ident_r = const.tile([P, P], f32r, name="ident_r")
            ind_sb = const.tile([P, MT * NQ], f32)
            qinv = const.tile([P, MT], f32)

            # chunk c of each transposed tensor has its own column band so a
            # packed 4-chunk PSUM bank evicts with one strided copy
            s_T = big.tile([P, KC * PAD_S], t_dt, name="s_T")
            q_T = big.tile([P, KC * PAD_P], t_dt, name="q_T")

            def sT(c):
                return s_T[:, c * PAD_S:(c + 1) * PAD_S]

            def qT(c):
                return q_T[:, c * PAD_P:(c + 1) * PAD_P]

            out_ps = outp.tile([NQ, WAY], f32)

            # ---- warmups: ACT tables + PE pipeline (no DMA deps) ----
            wtile = const.tile([P, P], f32, name="wtile")
            nc.gpsimd.memset(wtile, 1.0)
            wsq = small.tile([P, 1], f32, tag="snrm")
            nc.scalar.sqrt(wsq, wtile[:, 0:1])
            wps = tp4.tile([P, 4 * P], f32, tag="tp4")
            for i in range(N_WARM):
                nc.tensor.transpose(
                    wps[:, (i % 4) * P:(i % 4 + 1) * P], wtile, wtile
                )

            nev = 0

            def evict(out_ap, src_ap):
                nonlocal nev
                if nev % 2 == 0:
                    nc.vector.tensor_copy(out_ap, src_ap)
                else:
                    nc.scalar.copy(out_ap, src_ap)
                nev += 1

            def transpose_evict(x, T_all, T_pad, t, defer=False):
                """5 packed PE transposes of x into T_all's column bands."""
                isr = x.dtype == f32r
                idn = ident_r if isr else ident
                psA = tp4.tile([P, 4 * P], f32, tag="tp4", name=f"psA_{t}")
                psB = tp1.tile([P, P], f32, tag="tp1", name=f"psB_{t}")
                psAv = psA.bitcast(f32r) if isr else psA
                psBv = psB.bitcast(f32r) if isr else psB
                for c in range(4):
                    nc.tensor.transpose(
                        psAv[:, c * P:(c + 1) * P],
                        x[:, c * P:(c + 1) * P], idn)
                nc.tensor.transpose(psBv, x[:, 4 * P:5 * P], idn)
                out_ap = T_all[:, :4 * T_pad].rearrange(
                    "p (c n) -> p c n", c=4
                )[:, :, t * P:(t + 1) * P]

                def _ev():
                    evict(out_ap, psA.rearrange("p (c n) -> p c n", c=4))
                    evict(
                        T_all[:, 4 * T_pad + t * P:4 * T_pad + (t + 1) * P],
                        psB)
                if defer:
                    return _ev
                _ev()

            xs_s = [None] * ST
            xs_q = [None] * MT

            def s_dma(t, split=1):
                x = loads.tile([P, C], f32, tag="x_tile", name=f"sx{t}")
                h = P // split
                for i in range(split):
                    nc.sync.dma_start(
                        out=x[i * h:(i + 1) * h, :],
                        in_=s_d[t * P + i * h:t * P + (i + 1) * h, :])
                xs_s[t] = x

            def q_dma(m, split=1):
                x = loads.tile([P, C], f32, tag="x_tile", name=f"qx{m}")
                h = P // split
                for i in range(split):
                    nc.sync.dma_start(
                        out=x[i * h:(i + 1) * h, :],
                        in_=q_d[m * P + i * h:m * P + (i + 1) * h, :])
                xs_q[m] = x

            def s_prep(t, sq_on_dve=False, scale_on_dve=False,
                       defer=False):
                x = xs_s[t]
                sq = loads.tile([P, C], f32, tag="sq", name=f"ssq{t}")
                ssum = small.tile([P, 1], f32, tag="ssum")
                if sq_on_dve:
                    nc.vector.tensor_tensor_reduce(
                        sq, x, x, 1.0, 0.0,
                        mybir.AluOpType.mult, mybir.AluOpType.add, ssum)
                else:
                    nc.scalar.activation(sq, x, AF.Square, accum_out=ssum)
                snrm = small.tile([P, 1], f32, tag="snrm")
                nc.scalar.sqrt(snrm, ssum)
                sinv = small.tile([P, 1], f32, tag="sinv")
                nc.vector.reciprocal(sinv, snrm)
                s_n = loads.tile([P, C], f32r, tag="s_n", name=f"sn{t}")
                if scale_on_dve:
                    nc.vector.tensor_scalar_mul(s_n, x, sinv)
                else:
                    # NOTE: never gpsimd here - tensor_scalar on GPSIMD
                    # measures ~9.3us per [128,640] tile on real TRN2
                    nc.scalar.mul(s_n, x, sinv)
                return transpose_evict(s_n, s_T, PAD_S, t, defer=defer)

            def q_prep(m, defer=False):
                x = xs_q[m]
                ev = transpose_evict(x, q_T, PAD_P, m, defer=defer)
                sq = loads.tile([P, C], f32, tag="sq", name=f"qsq{m}")
                qsum = small.tile([P, 1], f32, tag="ssum")
                nc.scalar.activation(sq, x, AF.Square, accum_out=qsum)
                kn = small.tile([P, 1], f32, tag="snrm")
                # sqrt(k^2 * sum(q^2)) = k * |q|
                nc.scalar.activation(kn, qsum, AF.Sqrt, scale=float(k * k))
                nc.vector.reciprocal(qinv[:, m:m + 1], kn)
                return ev

            # ---- prologue: support tiles 0-3, queries 0-1 ----
            # DMA order: support first (its prep chain is the pace-setter),
            # then ident (first needed by real transposes), queries, ind.
            for t in range(4):
                s_dma(t)
            nc.sync.dma_start(out=ident, in_=ident_d)
            nc.vector.tensor_copy(ident_r, ident)
            q_dma(0)
            q_dma(1)
            nc.sync.dma_start(out=ind_sb, in_=ind_d)
            next_s = [4]

            def s_dma_ahead(upto):
                while next_s[0] <= min(upto, ST - 1):
                    s_dma(next_s[0])
                    next_s[0] += 1

            s_prep(0, scale_on_dve=True)
            s_prep(1, scale_on_dve=True)
            q_prep(0)
            s_prep(2, scale_on_dve=True)
            s_dma_ahead(5)
            s_prep(3, scale_on_dve=True)

            # s-prep schedule: pass w preps tiles 4w+4 .. 4w+7 (w<4)
            mxs = [None] * MT
            prev = [None, None]
            for w in range(WAY):
                for m in range(MT):
                    if w == 0:
                        if m + 2 < MT:
                            q_dma(m + 2)
                        if m + 1 < MT:
                            q_prep(m + 1)
                    if w < 4 and m in (1, 5, 9, 13):
                        t = 4 * (w + 1) + (m - 1) // 4
                        s_dma_ahead(t + 2)
                        s_prep(t, scale_on_dve=(t % 2 == 1))
                    if w == 0:
                        mxs[m] = mxp.tile([P, WAY * 8], f32, tag="mx",
                                          name=f"mx{m}")
                    psc = spp.tile([P, NW], f32, tag="psc",
                                   name=f"psc{m}_{w}")
                    for c in range(KC):
                        nc.tensor.matmul(
                            psc,
                            qT(c)[:, m * P:(m + 1) * P],
                            sT(c)[:, w * NW:(w + 1) * NW],
                            start=(c == 0),
                            stop=(c == KC - 1),
                        )
                    nc.vector.max(mxs[m][:, w * 8:(w + 1) * 8], psc)
                    if w == WAY - 1:
                        tsum = small.tile([P, WAY], f32, tag="tsum")
                        nc.vector.tensor_reduce(
                            tsum,
                            mxs[m].rearrange("p (w j) -> p w j", w=WAY)[:, :, :k],
                            axis=mybir.AxisListType.X,
                            op=mybir.AluOpType.add,
                        )
                        scaled = small.tile([P, WAY], f32, tag="scaled")
                        nc.scalar.mul(scaled, tsum, qinv[:, m:m + 1])
                        if prev[0] is not None:
                            nc.tensor.matmul(
                                out_ps,
                                ind_sb[:, prev[1] * NQ:(prev[1] + 1) * NQ],
                                prev[0], start=(prev[1] == 0), stop=False)
                        prev = [scaled, m]
            nc.tensor.matmul(
                out_ps, ind_sb[:, prev[1] * NQ:(prev[1] + 1) * NQ],
                prev[0], start=False, stop=True)
            out_sb = small.tile([NQ, WAY], f32, tag="out_sb")
            nc.scalar.copy(out_sb, out_ps)
            nc.sync.dma_start(out=out_d, in_=out_sb)

    nc.compile()
    return nc


def get_program(k: int):
    if k not in _prog_cache:
        _prog_cache[k] = _build(k)
    return _prog_cache[k]


def make_in_maps(input1: np.ndarray, input2: np.ndarray):
    """Shard full inputs into per-core input maps."""
    input1 = np.ascontiguousarray(np.asarray(input1), dtype=np.float32)
    input2 = np.ascontiguousarray(np.asarray(input2), dtype=np.float32)
    in_maps = []
    for core in range(N_CORES):
        b = core // 4
        qs = (core % 4) * NQ
        qe = min(Q, qs + NQ)
        nq = qe - qs
        qdat = input1[b].reshape(Q, HW, C)[qs:qe].reshape(-1, C)
        qfull = np.ones((PAD_P, C), np.float32)
        qfull[: nq * HW] = qdat
        sfull = np.ones((PAD_S, C), np.float32)
        sfull[:NS] = input2[b].reshape(NS, C)
        # indicator: patch row p of M-tile t belongs to query (t*128+p)//HW
        ind = np.zeros((P, MT * NQ), np.float32)
        g = np.arange(MT * P)
        j = g // HW
        valid = j < nq
        ind[g[valid] % P, (g[valid] // P) * NQ + j[valid]] = 1.0
        in_maps.append({"q": qfull, "s": sfull, "ind": ind,
                        "ident": np.eye(P, dtype=np.float32)})
    return in_maps


def gather_out(results) -> np.ndarray:
    out = np.zeros((B, Q, WAY), np.float32)
    for core in range(N_CORES):
        b = core // 4
        qs = (core % 4) * NQ
        n = min(Q, qs + NQ) - qs
        out[b, qs:qs + n] = results[core]["out"][:n]
    return out


def kernel(input1, input2, neighbor_k):
    k = int(np.asarray(neighbor_k))
    nc = get_program(k)
    in_maps = make_in_maps(input1, input2)
    # the axon-tunneled device occasionally reports a transient
    # "unrecoverable" state right after a previous process's teardown;
    # it recovers within seconds, so retry a couple of times
    import time
    last = None
    for attempt in range(3):
        try:
            res = run_bass_kernel_spmd(
                nc, in_maps, core_ids=list(range(N_CORES)))
            return gather_out(res.results)
        except Exception as e:  # noqa: BLE001
            last = e
            if attempt < 2:
                time.sleep(20.0 * (attempt + 1))
    raise last


# revision 32
# speedup vs baseline: 11970.2840x; 1.0088x over previous
"""TRN2 Bass kernel for nn_MetaBaseline (DN4-style local-descriptor kNN).

Reference computation (per batch b):
  q = normalize(input1[b].reshape(75, 100, 640), axis=-1)      # query patches
  s = normalize(input2[b].reshape(2500, 640), axis=-1)         # support descs
  scores = q_patches @ s.T                                     # [7500, 2500]
  per way group g (columns [500g, 500g+500)): top-k per row, mean,
  then sum over the 100 patches of each query -> out [75, 5].

Sharding: data-parallel over (b, query-quarter): 8 cores, each handles one
batch's quarter of queries (19 queries padded) with that batch's full
support replicated.

Per-core device program. Engines execute in emission order, so emission is
software-pipelined. The score loop is WAY-OUTER: pass w only needs support
descriptor tiles 0..4w+3, so score matmuls start as soon as the first four
support tiles are normalized+transposed; the remaining support prep streams
in the background during passes 0-3, and query prep (norm chain, packed PE
transposes, float32r eviction) is folded into pass 0 one tile ahead.
Top-8 per (patch, way) via DVE max straight from the PSUM score bank
(bank freed immediately after); pass 4 finishes each patch tile with a
strided top-k tensor_reduce, ACT scale by 1/(k*|q_patch|), and a small
fp32 indicator matmul accumulating per-query sums in PSUM -> [19, 5].
"""
import os
from contextlib import ExitStack

import numpy as np

import concourse.bass as bass  # noqa: F401
import concourse.mybir as mybir
import concourse.tile as tile
from concourse import bacc
from concourse.bass_utils import run_bass_kernel_spmd

# Problem geometry (hardcoded per contest rules)
B, Q, WAY, SHOT, H, W, C = 2, 75, 5, 5, 10, 10, 640
HW = H * W               # 100 patches per query / support image
NQ = 19                  # queries per core (4 cores x 19 = 76 >= 75)
MT = 15                  # patch M-tiles of 128 -> 1920 rows (1900 real)
PAD_P = MT * 128
NS = WAY * SHOT * HW     # 2500 support descriptors per batch
ST = 20                  # support tiles of 128 -> 2560 rows
PAD_S = ST * 128
KC = 5                   # C chunks of 128 (640 = 5*128)
P = 128
NW = SHOT * HW           # 500 support descriptors per way group
N_CORES = 8
N_WARM = int(os.environ.get("N_WARM", "14"))
BF16 = os.environ.get("BF16", "0") == "1"  # experimental: bf16 score operands

_prog_cache: dict[int, object] = {}


def _build(k: int):
    """Build + compile the per-core SPMD program for neighbor_k == k."""
    assert 1 <= k <= 8, f"neighbor_k={k} not supported (need 1..8)"
    nc = bacc.Bacc("TRN2", target_bir_lowering=False, debug=False)
    f32 = mybir.dt.float32
    f32r = mybir.dt.float32r
    t_dt = mybir.dt.bfloat16 if BF16 else f32r
    AF = mybir.ActivationFunctionType

    q_d = nc.dram_tensor("q", [PAD_P, C], f32, kind="ExternalInput").ap()
    s_d = nc.dram_tensor("s", [PAD_S, C], f32, kind="ExternalInput").ap()
    ind_d = nc.dram_tensor("ind", [P, MT * NQ], f32, kind="ExternalInput").ap()
    ident_d = nc.dram_tensor("ident", [P, P], f32, kind="ExternalInput").ap()
    out_d = nc.dram_tensor("out", [NQ, WAY], f32, kind="ExternalOutput").ap()

    with tile.TileContext(nc) as tc:
        with ExitStack() as ctx:
            const = ctx.enter_context(tc.tile_pool(name="const", bufs=1))
            big = ctx.enter_context(tc.tile_pool(name="big", bufs=1))
            loads = ctx.enter_context(tc.tile_pool(name="loads", bufs=6))
            small = ctx.enter_context(tc.tile_pool(name="small", bufs=4))
            mxp = ctx.enter_context(tc.tile_pool(name="mxp", bufs=MT))
            outp = ctx.enter_context(
                tc.tile_pool(name="outp", bufs=1, space="PSUM")
            )
            tp4 = ctx.enter_context(
                tc.tile_pool(name="tp4", bufs=2, space="PSUM")
            )
            tp1 = ctx.enter_context(
                tc.tile_pool(name="tp1", bufs=2, space="PSUM")
            )
            spp = ctx.enter_context(
                tc.tile_pool(name="spp", bufs=3, space="PSUM")
            )

            ident = const.tile([P, P], f32)
            ident_r = const.tile([P, P], f32r, name="ident_r")
            ind_sb = const.tile([P, MT * NQ], f32)
            qinv = const.tile([P, MT], f32)

            # chunk c of each transposed tensor has its own column band so a
            # packed 4-chunk PSUM bank evicts with one strided copy
            s_T = big.tile([P, KC * PAD_S], t_dt, name="s_T")
            q_T = big.tile([P, KC * PAD_P], t_dt, name="q_T")

            def sT(c):
                return s_T[:, c * PAD_S:(c + 1) * PAD_S]

            def qT(c):
                return q_T[:, c * PAD_P:(c + 1) * PAD_P]

            out_ps = outp.tile([NQ, WAY], f32)

            # ---- warmups: ACT tables + PE pipeline (no DMA deps) ----
            wtile = const.tile([P, P], f32, name="wtile")
            nc.gpsimd.memset(wtile, 1.0)
            wsq = small.tile([P, 1], f32, tag="snrm")
            nc.scalar.sqrt(wsq, wtile[:, 0:1])
            wps = tp4.tile([P, 4 * P], f32, tag="tp4")
            for i in range(N_WARM):
                nc.tensor.transpose(
                    wps[:, (i % 4) * P:(i % 4 + 1) * P], wtile, wtile
                )

            nev = 0

            def evict(out_ap, src_ap):
                nonlocal nev
                if nev % 2 == 0:
                    nc.vector.tensor_copy(out_ap, src_ap)
                else:
                    nc.scalar.copy(out_ap, src_ap)
                nev += 1

            def transpose_evict(x, T_all, T_pad, t, defer=False):
                """5 packed PE transposes of x into T_all's column bands."""
                isr = x.dtype == f32r
                idn = ident_r if isr else ident
                psA = tp4.tile([P, 4 * P], f32, tag="tp4", name=f"psA_{t}")
                psB = tp1.tile([P, P], f32, tag="tp1", name=f"psB_{t}")
                psAv = psA.bitcast(f32r) if isr else psA
                psBv = psB.bitcast(f32r) if isr else psB
                for c in range(4):
                    nc.tensor.transpose(
                        psAv[:, c * P:(c + 1) * P],
                        x[:, c * P:(c + 1) * P], idn)
                nc.tensor.transpose(psBv, x[:, 4 * P:5 * P], idn)
                out_ap = T_all[:, :4 * T_pad].rearrange(
                    "p (c n) -> p c n", c=4
                )[:, :, t * P:(t + 1) * P]

                def _ev():
                    evict(out_ap, psA.rearrange("p (c n) -> p c n", c=4))
                    evict(
                        T_all[:, 4 * T_pad + t * P:4 * T_pad + (t + 1) * P],
                        psB)
                if defer:
                    return _ev
                _ev()

            xs_s = [None] * ST
            xs_q = [None] * MT

            def s_dma(t, split=1):
                x = loads.tile([P, C], f32, tag="x_tile", name=f"sx{t}")
                h = P // split
                for i in range(split):
                    nc.sync.dma_start(
                        out=x[i * h:(i + 1) * h, :],
                        in_=s_d[t * P + i * h:t * P + (i + 1) * h, :])
                xs_s[t] = x

            def q_dma(m, split=1):
                x = loads.tile([P, C], f32, tag="x_tile", name=f"qx{m}")
                h = P // split
                for i in range(split):
                    nc.sync.dma_start(
                        out=x[i * h:(i + 1) * h, :],
                        in_=q_d[m * P + i * h:m * P + (i + 1) * h, :])
                xs_q[m] = x

            def s_prep(t, sq_on_dve=False, scale_on_dve=False,
                       defer=False):
                x = xs_s[t]
                sq = loads.tile([P, C], f32, tag="sq", name=f"ssq{t}")
                ssum = small.tile([P, 1], f32, tag="ssum")
                if sq_on_dve:
                    nc.vector.tensor_tensor_reduce(
                        sq, x, x, 1.0, 0.0,
                        mybir.AluOpType.mult, mybir.AluOpType.add, ssum)
                else:
                    nc.scalar.activation(sq, x, AF.Square, accum_out=ssum)
                snrm = small.tile([P, 1], f32, tag="snrm")
                nc.scalar.sqrt(snrm, ssum)
                sinv = small.tile([P, 1], f32, tag="sinv")
                nc.vector.reciprocal(sinv, snrm)
                s_n = loads.tile([P, C], f32r, tag="s_n", name=f"sn{t}")
                if scale_on_dve:
                    nc.vector.tensor_scalar_mul(s_n, x, sinv)
                else:
                    # NOTE: never gpsimd here - tensor_scalar on GPSIMD
                    # measures ~9.3us per [128,640] tile on real TRN2
                    nc.scalar.mul(s_n, x, sinv)
                return transpose_evict(s_n, s_T, PAD_S, t, defer=defer)

            def q_prep(m, defer=False):
                x = xs_q[m]
                ev = transpose_evict(x, q_T, PAD_P, m, defer=defer)
                sq = loads.tile([P, C], f32, tag="sq", name=f"qsq{m}")
                qsum = small.tile([P, 1], f32, tag="ssum")
                nc.scalar.activation(sq, x, AF.Square, accum_out=qsum)
                kn = small.tile([P, 1], f32, tag="snrm")
                # sqrt(k^2 * sum(q^2)) = k * |q|
                nc.scalar.activation(kn, qsum, AF.Sqrt, scale=float(k * k))
                nc.vector.reciprocal(qinv[:, m:m + 1], kn)
                return ev

            # ---- prologue: support tiles 0-3, queries 0-1 ----
            # DMA order: support first (its prep chain is the pace-setter),
            # then ident (first needed by real transposes), queries, ind.
            for t in range(4):
                s_dma(t)
            nc.sync.dma_start(out=ident, in_=ident_d)
            nc.vector.tensor_copy(ident_r, ident)
            q_dma(0)
            q_dma(1)
            nc.sync.dma_start(out=ind_sb, in_=ind_d)
            next_s = [4]

            def s_dma_ahead(upto):
                while next_s[0] <= min(upto, ST - 1):
                    s_dma(next_s[0])
                    next_s[0] += 1

            s_prep(0, scale_on_dve=True)
            s_prep(1, scale_on_dve=True)
            q_prep(0)
            s_prep(2, scale_on_dve=True)
            s_dma_ahead(5)
            s_prep(3, scale_on_dve=True)

            # s-prep schedule: pass w preps tiles 4w+4 .. 4w+7 (w<4)
            mxs = [None] * MT
            prev = [None, None]
            for w in range(WAY):
                for m in range(MT):
                    if w == 0:
                        if m + 2 < MT:
                            q_dma(m + 2)
                        if m + 1 < MT:
                            q_prep(m + 1)
                    if w < 4 and m in (1, 5, 9, 13):
                        t = 4 * (w + 1) + (m - 1) // 4
                        s_dma_ahead(t + 2)
                        s_prep(t, scale_on_dve=(t % 2 == 1))
                    if w == 0:
                        mxs[m] = mxp.tile([P, WAY * 8], f32, tag="mx",
                                          name=f"mx{m}")
                    psc = spp.tile([P, NW], f32, tag="psc",
                                   name=f"psc{m}_{w}")
                    for c in range(KC):
                        nc.tensor.matmul(
                            psc,
                            qT(c)[:, m * P:(m + 1) * P],
                            sT(c)[:, w * NW:(w + 1) * NW],
                            start=(c == 0),
                            stop=(c == KC - 1),
                        )
                    nc.vector.max(mxs[m][:, w * 8:(w + 1) * 8], psc)
                    if w == WAY - 1:
                        tsum = small.tile([P, WAY], f32, tag="tsum")
                        nc.vector.tensor_reduce(
                            tsum,
                            mxs[m].rearrange("p (w j) -> p w j", w=WAY)[:, :, :k],
                            axis=mybir.AxisListType.X,
                            op=mybir.AluOpType.add,
                        )
                        scaled = small.tile([P, WAY], f32, tag="scaled")
                        nc.scalar.mul(scaled, tsum, qinv[:, m:m + 1])
                        if prev[0] is not None:
                            nc.tensor.matmul(
                                out_ps,
                                ind_sb[:, prev[1] * NQ:(prev[1] + 1) * NQ],
                                prev[0], start=(prev[1] == 0), stop=False)
                        prev = [scaled, m]
            nc.tensor.matmul(
                out_ps, ind_sb[:, prev[1] * NQ:(prev[1] + 1) * NQ],
                prev[0], start=False, stop=True)
            out_sb = small.tile([NQ, WAY], f32, tag="out_sb")
            nc.scalar.copy(out_sb, out_ps)
            nc.sync.dma_start(out=out_d, in_=out_sb)

    nc.compile()
    return nc


def get_program(k: int):
    if k not in _prog_cache:
        _prog_cache[k] = _build(k)
    return _prog_cache[k]


def make_in_maps(input1: np.ndarray, input2: np.ndarray):
    """Shard full inputs into per-core input maps."""
    input1 = np.ascontiguousarray(np.asarray(input1), dtype=np.float32)
    input2 = np.ascontiguousarray(np.asarray(input2), dtype=np.float32)
    in_maps = []
    for core in range(N_CORES):
        b = core // 4
        qs = (core % 4) * NQ
        qe = min(Q, qs + NQ)
        nq = qe - qs
        qdat = input1[b].reshape(Q, HW, C)[qs:qe].reshape(-1, C)
        qfull = np.ones((PAD_P, C), np.float32)
        qfull[: nq * HW] = qdat
        sfull = np.ones((PAD_S, C), np.float32)
        sfull[:NS] = input2[b].reshape(NS, C)
        # indicator: patch row p of M-tile t belongs to query (t*128+p)//HW
        ind = np.zeros((P, MT * NQ), np.float32)
        g = np.arange(MT * P)
        j = g // HW
        valid = j < nq
        ind[g[valid] % P, (g[valid] // P) * NQ + j[valid]] = 1.0
        in_maps.append({"q": qfull, "s": sfull, "ind": ind,
                        "ident": np.eye(P, dtype=np.float32)})
    return in_maps


def gather_out(results) -> np.ndarray:
    out = np.zeros((B, Q, WAY), np.float32)
    for core in range(N_CORES):
        b = core // 4
        qs = (core % 4) * NQ
        n = min(Q, qs + NQ) - qs
        out[b, qs:qs + n] = results[core]["out"][:n]
    return out


def kernel(input1, input2, neighbor_k):
    k = int(np.asarray(neighbor_k))
    nc = get_program(k)
    in_maps = make_in_maps(input1, input2)
    # the axon-tunneled device occasionally reports a transient
    # "unrecoverable" state right after a previous process's teardown;
    # it recovers within seconds, so retry a couple of times
    import time
    last = None
    for attempt in range(3):
        try:
            res = run_bass_kernel_spmd(
                nc, in_maps, core_ids=list(range(N_CORES)))
            return gather_out(res.results)
        except Exception as e:  # noqa: BLE001
            last = e
            if attempt < 2:
                time.sleep(20.0 * (attempt + 1))
    raise last
